# revision 7
# baseline (speedup 1.0000x reference)
"""Causal multi-head attention (B=2, S=2048, D=1024, H=16, HD=64) on 8 NeuronCores.

Sharding: core c = 4*b + g handles batch b (2-way data parallel) and head
group g (4-way tensor parallel over the 16 heads, 4 heads per core).
Each core computes its 4 heads' attention plus the partial output
projection (columns of Wo for its heads); the host sums the 4 partials
per batch ("row-parallel" reduction) to produce the full output.

v3 notes (cost-model driven):
  - All I/O and matmul operands are fp16: halves DMA bytes (the TimelineSim
    DMA_ENGINES resource is aggregate-bandwidth serialized) and keeps every
    matmul at 1.0 cycles/row regardless of moving width.
  - Input DMAs are merged into few large transfers (HWDGE charges ~630ns of
    a single serialized resource per dma_start); output stages into
    [128, 2, 512] tiles so each 128-row band is one DMA.
  - engine balance: exp must run on the scalar engine (the only exp-capable
    engine) and is ~75us of an ~115us-PE kernel, so every other elementwise
    op is kept off it mid-kernel: V copies go to the scalar engine only in
    the early (projection) window where it idles, triangle masks run on
    GpSimd, softmax normalization multiplies read the reciprocal row through
    a partition-broadcast AP (no materialized broadcast), and output-
    projection drains split DVE/scalar per half so the two halves of each
    row band drain in parallel.
  - X is fed transposed (xt = X[b].T, [D,S]) so the d-contraction of the
    QKV projections has d on SBUF partitions; Q,K are produced transposed
    so scores are computed transposed (S^T[k,q]) and P@V needs no
    transposes; V carries a ones column so P@V also yields the softmax
    denominator; softmax skips max-subtraction (scores/8 ~ N(0,1)).
  - K3/V3/Q3 projections are deferred into the exp-bound window so the PE
    has independent work while the scalar engine drains the chunk-2/3
    score backlog; the last output projection (chunk 1) depends only on
    early-finished attention so the tail is pure drain+DMA.
"""

import numpy as np

import concourse.mybir as mybir
from concourse import bacc
from concourse.tile import TileContext
from concourse.bass_utils import run_bass_kernel_spmd
from concourse.masks import make_upper_triangular

F32 = mybir.dt.float32
FP16 = mybir.dt.float16
Exp = mybir.ActivationFunctionType.Exp
Alu = mybir.AluOpType

B, S, D, H, HD = 2, 2048, 1024, 16, 64
GH = 4            # heads per core
GD = GH * HD      # 256 features per core
N_CORES = 8


def _build():
    nc = bacc.Bacc("TRN2", target_bir_lowering=False, name="mha_tp")
    xt_d = nc.dram_tensor("xt", [D, S], FP16, kind="ExternalInput")
    wq_d = nc.dram_tensor("wqT", [D, GD], FP16, kind="ExternalInput")
    wk_d = nc.dram_tensor("wkT", [D, GD], FP16, kind="ExternalInput")
    wv_d = nc.dram_tensor("wvT", [D, GD], FP16, kind="ExternalInput")
    wo_d = nc.dram_tensor("woT", [GD, D], FP16, kind="ExternalInput")
    out_d = nc.dram_tensor("out", [S, D], FP16, kind="ExternalOutput")

    with TileContext(nc) as tc:
        with (
            tc.tile_pool(name="per", bufs=1) as per,
            tc.tile_pool(name="pt", bufs=10) as ptp,
            tc.tile_pool(name="wk1", bufs=2) as wk1,
            tc.tile_pool(name="wk2", bufs=6) as wk2,
            tc.tile_pool(name="ps_a", bufs=2, space="PSUM") as ps_a,
            tc.tile_pool(name="ps_o", bufs=2, space="PSUM") as ps_o,
            tc.tile_pool(name="ps_c", bufs=2, space="PSUM") as ps_c,
        ):
            xt = per.tile([128, 8, S], FP16)       # X^T, d-tile major
            wo = per.tile([128, 2, D], FP16)       # Wo^T for our head cols
            qt = per.tile([128, 2, S], FP16)       # Q^T (2 heads per tile)
            kt = per.tile([128, 2, S], FP16)
            vaug = per.tile([128, 16, 4 * (HD + 1)], FP16)  # V + ones col per head
            ctxn = per.tile([128, 2, S], FP16)     # normalized ctx^T
            tri = per.tile([128, 128], FP16)       # tri[kk,c]=1 iff kk<=c
            wq = per.tile([128, 8, GD], FP16)
            wk = per.tile([128, 8, GD], FP16)
            wv = per.tile([128, 8, GD], FP16)

            make_upper_triangular(nc, tri[:, :], val=1.0, diag=True)

            # ---- input DMA waves: few large transfers, ordered so the
            # ---- first Q/K projections and chunk-0 attention unblock early.
            # dram rows are d-features: row 128*a + p maps to SBUF
            # partition p, d-tile slot a.
            def dview(t, r0, r1, c0, c1):
                return t[r0:r1, c0:c1].rearrange("(a p) c -> p a c", p=128)

            nc.sync.dma_start(wq[:, 0:2, :], dview(wq_d, 0, 256, 0, GD))
            nc.scalar.dma_start(xt[:, 0:2, 0:512], dview(xt_d, 0, 256, 0, 512))
            nc.sync.dma_start(wq[:, 2:8, :], dview(wq_d, 256, 1024, 0, GD))
            nc.scalar.dma_start(xt[:, 2:8, 0:512], dview(xt_d, 256, 1024, 0, 512))
            nc.sync.dma_start(wk[:, :, :], dview(wk_d, 0, 1024, 0, GD))
            nc.scalar.dma_start(xt[:, :, 512:1024], dview(xt_d, 0, 1024, 512, 1024))
            nc.sync.dma_start(wv[:, :, :], dview(wv_d, 0, 1024, 0, GD))
            nc.scalar.dma_start(xt[:, :, 1024:1536], dview(xt_d, 0, 1024, 1024, 1536))
            nc.sync.dma_start(xt[:, :, 1536:2048], dview(xt_d, 0, 1024, 1536, 2048))
            nc.sync.dma_start(wo[:, :, :], dview(wo_d, 0, 256, 0, D))

            def emit_qk(w_t, dst, sc):
                for dp in range(2):
                    ps = ps_a.tile([128, 512], F32, tag="blk")
                    for dt in range(8):
                        nc.tensor.matmul(
                            ps[:, :],
                            w_t[:, dt, 128 * dp:128 * dp + 128],
                            xt[:, dt, 512 * sc:512 * sc + 512],
                            start=(dt == 0), stop=(dt == 7),
                        )
                    nc.vector.tensor_copy(dst[:, dp, 512 * sc:512 * sc + 512], ps[:, :])

            def emit_v(sc, copy_eng="scalar"):
                for st in range(4 * sc, 4 * sc + 4):
                    psv = ps_a.tile([128, 256], F32, tag="blk")
                    for dt in range(8):
                        nc.tensor.matmul(
                            psv[:, :],
                            xt[:, dt, 128 * st:128 * st + 128],
                            wv[:, dt, :],
                            start=(dt == 0), stop=(dt == 7),
                        )
                    v_dst = vaug[:, st, :].rearrange("p (h c) -> p h c", c=HD + 1)
                    if copy_eng == "scalar":
                        nc.scalar.copy(
                            v_dst[:, :, 0:HD],
                            psv.rearrange("p (h c) -> p h c", c=HD),
                        )
                    else:
                        nc.vector.tensor_copy(
                            v_dst[:, :, 0:HD],
                            psv.rearrange("p (h c) -> p h c", c=HD),
                        )
                    # ones column: x*0+1 through DVE so the write is rounded
                    nc.vector.tensor_scalar(
                        v_dst[:, :, HD:HD + 1], psv[:, 0:4], 0.0, 1.0,
                        op0=Alu.mult, op1=Alu.add,
                    )

            def emit_head_pair(qc, i):
                """Heads hA=2i (PE rows 0-63) and hB=2i+1 (rows 64-127): their
                score matmuls are emitted alternating; the two heads' chains
                stay independent so exp/ctx pipeline across heads."""
                hA, hB = 2 * i, 2 * i + 1
                heads = ((hA, 0), (hB, 64))
                ctxs = {}
                pts = {h: [] for h, _ in heads}
                packs = [
                    (896, ((0, 0, 512), (1, 512, 384))),
                    (384, ((3, 0, 128), (2, 128, 256))),
                ]
                for h, qo in heads:
                    ctx_t = ps_c.tile([65, 512], F32, tag="ctx")
                    ctxs[h] = ctx_t
                # diagonal strips
                for width, parts in packs:
                    sps = {}
                    for h, qo in heads:
                        sp_t = ps_a.tile([128, 1024], F32, tag="blk")
                        sps[h] = sp_t
                    for j, o, w in parts:
                        k_t = 4 * qc + j
                        for h, qo in heads:
                            nc.tensor.matmul(
                                sps[h][:, o:o + w],
                                kt[qo:qo + 64, i, 128 * k_t:128 * k_t + 128],
                                qt[qo:qo + 64, i, 512 * qc + 128 * j:512 * qc + 128 * j + w],
                                start=True, stop=True,
                            )
                    for h, qo in heads:
                        pt_p = ptp.tile([128, 1024], FP16, tag="pt")
                        nc.scalar.activation(pt_p[:, :width], sps[h][:, :width], Exp, scale=0.125)
                        for ii, (j, o, w) in enumerate(parts):
                            eng = nc.gpsimd if ii == 0 else nc.vector
                            eng.tensor_mul(
                                pt_p[:, o:o + 128], pt_p[:, o:o + 128], tri[:, :]
                            )
                        pts[h].append((pt_p, parts))
                # full blocks (2 k-tiles per tile), pairwise
                for blk in range(2 * qc):
                    sps = {}
                    for h, qo in heads:
                        sp_t = ps_a.tile([128, 1024], F32, tag="blk")
                        sps[h] = sp_t
                    for j2 in range(2):
                        k_t = 2 * blk + j2
                        for h, qo in heads:
                            nc.tensor.matmul(
                                sps[h][:, 512 * j2:512 * j2 + 512],
                                kt[qo:qo + 64, i, 128 * k_t:128 * k_t + 128],
                                qt[qo:qo + 64, i, 512 * qc:512 * qc + 512],
                                start=True, stop=True,
                            )
                    for h, qo in heads:
                        pt_b = ptp.tile([128, 1024], FP16, tag="pt")
                        nc.scalar.activation(pt_b[:, :], sps[h][:, :], Exp, scale=0.125)
                        pts[h].append((pt_b, ((None, 0, 512), (None, 512, 512))))
                # ctx accumulation per head
                for h, qo in heads:
                    ctx = ctxs[h]
                    ctx_mms = []
                    for bi, (pt_t, parts) in enumerate(pts[h]):
                        for pj, (j, o, w) in enumerate(parts):
                            if bi < 2:          # diagonal strip tiles
                                k_t, co = 4 * qc + j, 128 * j
                            else:               # full block tiles
                                k_t, co = 2 * (bi - 2) + pj, 0
                            ctx_mms.append((pt_t, k_t, o, w, co))
                    for n, (pt_t, k_t, o, w, co) in enumerate(ctx_mms):
                        nc.tensor.matmul(
                            ctx[:, co:co + w],
                            vaug[:, k_t, 65 * h:65 * h + 65],
                            pt_t[:, o:o + w],
                            start=(n == 0), stop=(n == len(ctx_mms) - 1),
                        )
                # normalize both heads: l rows to SBUF, one recip, broadcast,
                # scale each head's PSUM ctx into ctxn
                for n, (h, qo) in enumerate(heads):
                    l_sb = wk1.tile([1, 512], F32, tag="lrow")
                    nc.vector.tensor_copy(l_sb[:, :], ctxs[h][64:65, :])
                    r_sb = wk1.tile([1, 512], F32, tag="rrow")
                    nc.vector.reciprocal_approx_fast(r_sb[:, :], l_sb[:, :])
                    rb = wk1.tile([64, 512], F32, tag="rb")
                    nc.gpsimd.partition_broadcast(rb[:, :], r_sb[:1, :], channels=64)
                    nc.vector.tensor_mul(
                        ctxn[qo:qo + 64, i, 512 * qc:512 * qc + 512],
                        ctxs[h][0:64, :], rb[:, :],
                    )

            def emit_outproj(qc, split=False):
                # bias is added on the host during the unshard sum; `split`
                # drains the two halves of each row band on DVE + scalar in
                # parallel (used where the scalar engine has slack)
                for st in range(4 * qc, 4 * qc + 4):
                    ob = wk2.tile([128, 2, 512], FP16, tag="ob")
                    for oc in range(2):
                        pso = ps_o.tile([128, 512], F32, tag="po")
                        for dp in range(2):
                            nc.tensor.matmul(
                                pso[:, :],
                                ctxn[:, dp, 128 * st:128 * st + 128],
                                wo[:, dp, 512 * oc:512 * oc + 512],
                                start=(dp == 0), stop=(dp == 1),
                            )
                        if split and oc == 1:
                            nc.scalar.copy(ob[:, oc, :], pso[:, :])
                        else:
                            nc.vector.tensor_copy(ob[:, oc, :], pso[:, :])
                    nc.sync.dma_start(
                        out_d[128 * st:128 * st + 128, :],
                        ob.rearrange("p a b -> p (a b)"),
                    )

            # ---- projection waves with chunk-0/1 attention folded in ----
            emit_qk(wq, qt, 0)
            emit_qk(wq, qt, 1)
            emit_v(0)
            emit_v(1)
            emit_qk(wk, kt, 0)
            emit_head_pair(0, 0)
            emit_head_pair(0, 1)
            emit_qk(wk, kt, 1)
            emit_qk(wq, qt, 2)
            emit_v(2)
            emit_head_pair(1, 0)
            emit_qk(wk, kt, 2)
            # ---- exp-bound window: weave remaining projections + outprojs
            emit_head_pair(2, 0)
            emit_qk(wq, qt, 3)
            emit_head_pair(2, 1)
            emit_v(3, copy_eng="vector")
            emit_qk(wk, kt, 3)
            emit_outproj(0)
            emit_head_pair(3, 0)
            emit_head_pair(1, 1)
            emit_head_pair(3, 1)
            emit_outproj(2, split=True)
            emit_outproj(3, split=True)
            emit_outproj(1, split=True)
    nc.compile()
    return nc


_NC = None


def _get_nc():
    global _NC
    if _NC is None:
        _NC = _build()
    return _NC


def _in_maps(x, wq, wk, wv, wo):
    xts = [np.ascontiguousarray(x[b].T).astype(np.float16) for b in range(B)]
    in_maps = []
    for c in range(N_CORES):
        b, g = c // 4, c % 4
        sl = slice(GD * g, GD * g + GD)
        in_maps.append({
            "xt": xts[b],
            "wqT": np.ascontiguousarray(wq[sl, :].T).astype(np.float16),
            "wkT": np.ascontiguousarray(wk[sl, :].T).astype(np.float16),
            "wvT": np.ascontiguousarray(wv[sl, :].T).astype(np.float16),
            "woT": np.ascontiguousarray(wo[:, sl].T).astype(np.float16),
        })
    return in_maps


def kernel(**inputs):
    x = np.asarray(inputs["inputs"], dtype=np.float32)
    wq = np.asarray(inputs["Wq"], dtype=np.float32)
    wk = np.asarray(inputs["Wk"], dtype=np.float32)
    wv = np.asarray(inputs["Wv"], dtype=np.float32)
    wo = np.asarray(inputs["Wo"], dtype=np.float32)
    bo = np.asarray(inputs["bo"], dtype=np.float32)

    nc = _get_nc()
    res = run_bass_kernel_spmd(nc, _in_maps(x, wq, wk, wv, wo),
                               core_ids=list(range(N_CORES)))
    out = np.empty((B, S, D), np.float32)
    for b in range(B):
        acc = res.results[4 * b + 0]["out"].astype(np.float32)
        for g in range(1, 4):
            acc = acc + res.results[4 * b + g]["out"].astype(np.float32)
        out[b] = acc + bo
    return out


# revision 16
# speedup vs baseline: 1.1520x; 1.1520x over previous
"""Causal multi-head attention (B=2, S=2048, D=1024, H=16, HD=64) on 8 NeuronCores.

Sharding: core c = 4*b + g handles batch b (2-way data parallel) and head
group g (4-way tensor parallel over the 16 heads, 4 heads per core).
Each core computes its 4 heads' attention plus the partial output
projection (columns of Wo for its heads); the host sums the 4 partials
per batch ("row-parallel" reduction) to produce the full output.

Device layout notes:
  - All I/O and matmul operands are fp16: halves DMA bytes (the TimelineSim
    DMA_ENGINES resource is aggregate-bandwidth serialized) and keeps every
    matmul at 1.0 cycles/row regardless of moving width.
  - Input DMAs are merged into few large transfers (HWDGE charges ~630ns of
    a single serialized resource per dma_start); output stages into
    [128, 2, 512] tiles so each 128-row band is one DMA.
  - X is fed transposed (xt = X[b].T, [D,S]) so the d-contraction of the
    QKV projections has d on SBUF partitions; Q,K are produced transposed
    so scores are computed transposed (S^T[k,q]) and P@V needs no
    transposes; V carries a ones column so P@V also yields the softmax
    denominator; softmax skips max-subtraction (scores/8 ~ N(0,1)).
  - exp runs on the scalar engine only (the sole exp engine, ~75us of an
    ~115us-PE kernel): that queue is kept free of other work mid-kernel.
  - chunks 0 and 1 of the attention run inside the DMA-bound projection
    window; the exp-heavy chunks 3 and 2 run interleaved afterwards.
"""

import numpy as np

import concourse.mybir as mybir
from concourse import bacc
from concourse.tile import TileContext
from concourse.bass_utils import run_bass_kernel_spmd
from concourse.masks import make_identity, make_upper_triangular

F32 = mybir.dt.float32
FP16 = mybir.dt.float16
I16 = mybir.dt.int16
Exp = mybir.ActivationFunctionType.Exp
Alu = mybir.AluOpType

B, S, D, H, HD = 2, 2048, 1024, 16, 64
GH = 4            # heads per core
GD = GH * HD      # 256 features per core
N_CORES = 8

# Schedule / engine-assignment knobs (overridable for experiments).
KNOBS = dict(
    # emission order: projections with chunk-0/1 attention folded in, then
    # the exp-heavy chunk-2/3 pairs with output-projection slots woven in
    # at pair boundaries (PE filler while the scalar engine drains exps)
    order=[
        ("qk", "q", 0), ("qk", "q", 1), ("v", 0), ("v", 1), ("qk", "k", 0),
        ("pair", 0, 0), ("pair", 0, 1),
        ("qk", "q", 2), ("qk", "k", 1), ("v", 2),
        ("pair", 1, 0),
        ("qk", "q", 3), ("qk", "k", 2), ("v", 3), ("qk", "k", 3),
        ("oslot", 0, 0), ("pair", 1, 1),
        ("oslot", 0, 1), ("pair", 3, 0),
        ("oslot", 0, 2), ("oslot", 1, 4), ("pair", 2, 0),
        ("oslot", 0, 3), ("oslot", 1, 5), ("pair", 3, 1),
        ("oslot", 1, 6), ("oslot", 1, 7), ("pair", 2, 1),
        ("oproj", 3), ("oproj", 2),
    ],
    # which oproj chunks split their drains across DVE+scalar: {qc: mode}
    # mode: "dve" (all DVE), "split" (oc0 DVE, oc1 scalar), "scalar", "alt"
    drain={0: "dve", 3: "split", 2: "split", 1: "split"},
    pt_bufs=10,
    ps_a_bufs=2,
    pack_order="21",   # small diagonal pack first: its short exps free the
                       # score PSUM ring faster at pair starts
    dma_plan="fine",
    # pairs whose l-row copies go to the scalar engine (set of (qc,i))
    lcopy_scalar=set(),
    recip_psum=False,   # reciprocal reads the PSUM l-row directly
    # pairs whose odd block-exps run on DVE via the Schraudolph bit trick
    exp_dve=set(),
    interleave_ctx=False,  # weave ctx MMs between block score MMs
    mask_mm=False,      # accumulate -1e3 mask via PE instead of tri-muls
    warmup=0,           # dummy PE matmuls at t=0 to warm the pstate ramp
    dma_first_fine=False,
)


def _build(knobs=None):
    kn = dict(KNOBS)
    if knobs:
        kn.update(knobs)

    nc = bacc.Bacc("TRN2", target_bir_lowering=False, name="mha_tp")
    xt_d = nc.dram_tensor("xt", [D, S], FP16, kind="ExternalInput")
    wq_d = nc.dram_tensor("wqT", [D, GD], FP16, kind="ExternalInput")
    wk_d = nc.dram_tensor("wkT", [D, GD], FP16, kind="ExternalInput")
    wv_d = nc.dram_tensor("wvT", [D, GD], FP16, kind="ExternalInput")
    wo_d = nc.dram_tensor("woT", [GD, D], FP16, kind="ExternalInput")
    out_d = nc.dram_tensor("out", [S, D], FP16, kind="ExternalOutput")

    with TileContext(nc) as tc:
        with (
            tc.tile_pool(name="per", bufs=1) as per,
            tc.tile_pool(name="pt", bufs=kn["pt_bufs"]) as ptp,
            tc.tile_pool(name="wk1", bufs=1) as wk1,
            tc.tile_pool(name="wk2", bufs=6) as wk2,
            tc.tile_pool(name="ps_a", bufs=kn["ps_a_bufs"], space="PSUM") as ps_a,
            tc.tile_pool(name="ps_o", bufs=2, space="PSUM") as ps_o,
            tc.tile_pool(name="ps_c", bufs=2, space="PSUM") as ps_c,
        ):
            xt = per.tile([128, 8, S], FP16)       # X^T, d-tile major
            wo = per.tile([128, 2, D], FP16)       # Wo^T for our head cols
            qt = per.tile([128, 2, S], FP16)       # Q^T (2 heads per tile)
            kt = per.tile([128, 2, S], FP16)
            vaug = per.tile([128, 16, 4 * (HD + 1)], FP16)  # V + ones col per head
            ctxn = per.tile([128, 2, S], FP16)     # normalized ctx^T
            tri = per.tile([128, 128], FP16)       # tri[kk,c]=1 iff kk<=c
            wq = per.tile([128, 8, GD], FP16)
            wk = per.tile([128, 8, GD], FP16)
            wv = per.tile([128, 8, GD], FP16)

            make_upper_triangular(nc, tri[:, :], val=1.0, diag=True)
            if kn["mask_mm"]:
                ident = per.tile([128, 128], FP16)
                trim = per.tile([128, 128], FP16)
                make_identity(nc, ident[:, :])
                # trim[kk, c] = -1e3 where kk > c (masked region), else 0:
                # (tri - 1) * 1e3
                nc.vector.tensor_scalar(
                    trim[:, :], tri[:, :], -1.0, 1e3,
                    op0=Alu.add, op1=Alu.mult,
                )
            if kn["warmup"]:
                wps = ps_o.tile([128, 512], F32, tag="po")
                for wi in range(kn["warmup"]):
                    nc.tensor.matmul(wps[:, 0:128], tri[:, :], tri[:, :],
                                     start=(wi == 0),
                                     stop=(wi == kn["warmup"] - 1))

            # ---- input DMA waves. dram rows are d-features: row 128*a + p
            # maps to SBUF partition p, d-tile slot a.
            def dview(t, r0, r1, c0, c1):
                return t[r0:r1, c0:c1].rearrange("(a p) c -> p a c", p=128)

            if kn["dma_plan"] == "fine":
                if kn["dma_first_fine"]:
                    nc.sync.dma_start(wq[:, 0:1, :], dview(wq_d, 0, 128, 0, GD))
                    nc.scalar.dma_start(xt[:, 0:1, 0:512], dview(xt_d, 0, 128, 0, 512))
                    nc.sync.dma_start(wq[:, 1:2, :], dview(wq_d, 128, 256, 0, GD))
                    nc.scalar.dma_start(xt[:, 1:2, 0:512], dview(xt_d, 128, 256, 0, 512))
                else:
                    nc.sync.dma_start(wq[:, 0:2, :], dview(wq_d, 0, 256, 0, GD))
                    nc.scalar.dma_start(xt[:, 0:2, 0:512], dview(xt_d, 0, 256, 0, 512))
                nc.sync.dma_start(wq[:, 2:5, :], dview(wq_d, 256, 640, 0, GD))
                nc.scalar.dma_start(xt[:, 2:5, 0:512], dview(xt_d, 256, 640, 0, 512))
                nc.sync.dma_start(wq[:, 5:8, :], dview(wq_d, 640, 1024, 0, GD))
                nc.scalar.dma_start(xt[:, 5:8, 0:512], dview(xt_d, 640, 1024, 0, 512))
                nc.sync.dma_start(xt[:, 0:4, 512:1024], dview(xt_d, 0, 512, 512, 1024))
                nc.scalar.dma_start(xt[:, 4:8, 512:1024], dview(xt_d, 512, 1024, 512, 1024))
                nc.sync.dma_start(wv[:, :, :], dview(wv_d, 0, 1024, 0, GD))
                nc.scalar.dma_start(wk[:, :, :], dview(wk_d, 0, 1024, 0, GD))
                nc.scalar.dma_start(xt[:, :, 1024:1536], dview(xt_d, 0, 1024, 1024, 1536))
                nc.sync.dma_start(xt[:, :, 1536:2048], dview(xt_d, 0, 1024, 1536, 2048))
                nc.sync.dma_start(wo[:, :, :], dview(wo_d, 0, 256, 0, D))
            else:
                nc.sync.dma_start(wq[:, 0:2, :], dview(wq_d, 0, 256, 0, GD))
                nc.scalar.dma_start(xt[:, 0:2, 0:512], dview(xt_d, 0, 256, 0, 512))
                nc.sync.dma_start(wq[:, 2:8, :], dview(wq_d, 256, 1024, 0, GD))
                nc.scalar.dma_start(xt[:, 2:8, 0:512], dview(xt_d, 256, 1024, 0, 512))
                nc.sync.dma_start(wk[:, :, :], dview(wk_d, 0, 1024, 0, GD))
                nc.scalar.dma_start(xt[:, :, 512:1024], dview(xt_d, 0, 1024, 512, 1024))
                nc.sync.dma_start(wv[:, :, :], dview(wv_d, 0, 1024, 0, GD))
                nc.scalar.dma_start(xt[:, :, 1024:1536], dview(xt_d, 0, 1024, 1024, 1536))
                nc.sync.dma_start(xt[:, :, 1536:2048], dview(xt_d, 0, 1024, 1536, 2048))
                nc.sync.dma_start(wo[:, :, :], dview(wo_d, 0, 256, 0, D))

            def emit_qk(w_t, dst, sc):
                for dp in range(2):
                    ps = ps_a.tile([128, 512], F32, tag="blk")
                    for dt in range(8):
                        nc.tensor.matmul(
                            ps[:, :],
                            w_t[:, dt, 128 * dp:128 * dp + 128],
                            xt[:, dt, 512 * sc:512 * sc + 512],
                            start=(dt == 0), stop=(dt == 7),
                        )
                    nc.vector.tensor_copy(dst[:, dp, 512 * sc:512 * sc + 512], ps[:, :])

            def emit_v(sc):
                for st in range(4 * sc, 4 * sc + 4):
                    psv = ps_a.tile([128, 256], F32, tag="blk")
                    for dt in range(8):
                        nc.tensor.matmul(
                            psv[:, :],
                            xt[:, dt, 128 * st:128 * st + 128],
                            wv[:, dt, :],
                            start=(dt == 0), stop=(dt == 7),
                        )
                    v_dst = vaug[:, st, :].rearrange("p (h c) -> p h c", c=HD + 1)
                    nc.vector.tensor_copy(
                        v_dst[:, :, 0:HD],
                        psv.rearrange("p (h c) -> p h c", c=HD),
                    )
                    # ones column: x*0+1 through DVE so the write is rounded
                    nc.vector.tensor_scalar(
                        v_dst[:, :, HD:HD + 1], psv[:, 0:4], 0.0, 1.0,
                        op0=Alu.mult, op1=Alu.add,
                    )

            def emit_head_pair(qc, i, filler=None):
                """Heads hA=2i (PE rows 0-63) and hB=2i+1 (rows 64-127): their
                score matmuls are emitted alternating so the two heads' chains
                pipeline through exp/ctx independently. With interleave_ctx,
                each block's P@V accumulation is emitted one block behind the
                scores so the PE has work while exps drain."""
                hA, hB = 2 * i, 2 * i + 1
                heads = ((hA, 0), (hB, 64))
                ctxs = {}
                pts = {h: [] for h, _ in heads}
                packs = [
                    (896, ((0, 0, 512), (1, 512, 384))),
                    (384, ((3, 0, 128), (2, 128, 256))),
                ]
                if kn["pack_order"] == "21":
                    packs = packs[::-1]
                for h, qo in heads:
                    ctx_t = ps_c.tile([65, 512], F32, tag="ctx")
                    ctxs[h] = ctx_t

                n_ctx = {h: 2 * (2 + 2 * qc) for h, _ in heads}
                ctx_done = {h: 0 for h, _ in heads}

                def emit_ctx(h, upto):
                    ctx = ctxs[h]
                    jobs = []
                    for bi, (pt_t, parts) in enumerate(pts[h][:upto]):
                        for pj, (j, o, w) in enumerate(parts):
                            if bi < 2:          # diagonal strip tiles
                                k_t, co = 4 * qc + j, 128 * j
                            else:               # full block tiles
                                k_t, co = 2 * (bi - 2) + pj, 0
                            jobs.append((pt_t, k_t, o, w, co))
                    total = n_ctx[h]
                    for n in range(ctx_done[h], len(jobs)):
                        pt_t, k_t, o, w, co = jobs[n]
                        nc.tensor.matmul(
                            ctx[:, co:co + w],
                            vaug[:, k_t, 65 * h:65 * h + 65],
                            pt_t[:, o:o + w],
                            start=(n == 0), stop=(n == total - 1),
                            skip_group_check=True,
                        )
                    ctx_done[h] = len(jobs)

                # diagonal strips
                for width, parts in packs:
                    sps = {}
                    for h, qo in heads:
                        sp_t = ps_a.tile([128, 1024], F32, tag="blk")
                        sps[h] = sp_t
                    for j, o, w in parts:
                        k_t = 4 * qc + j
                        for h, qo in heads:
                            nc.tensor.matmul(
                                sps[h][:, o:o + w],
                                kt[qo:qo + 64, i, 128 * k_t:128 * k_t + 128],
                                qt[qo:qo + 64, i, 512 * qc + 128 * j:512 * qc + 128 * j + w],
                                start=True, stop=True,
                            )
                    for h, qo in heads:
                        pt_p = ptp.tile([128, 1024], FP16, tag="pt")
                        nc.scalar.activation(pt_p[:, :width], sps[h][:, :width], Exp, scale=0.125)
                        for ii, (j, o, w) in enumerate(parts):
                            eng = nc.vector if ii == 0 else nc.gpsimd
                            eng.tensor_mul(
                                pt_p[:, o:o + 128], pt_p[:, o:o + 128], tri[:, :]
                            )
                        pts[h].append((pt_p, parts))
                if filler is not None:
                    filler()
                # full blocks (2 k-tiles per tile), pairwise
                for blk in range(2 * qc):
                    sps = {}
                    for h, qo in heads:
                        sp_t = ps_a.tile([128, 1024], F32, tag="blk")
                        sps[h] = sp_t
                    for j2 in range(2):
                        k_t = 2 * blk + j2
                        for h, qo in heads:
                            nc.tensor.matmul(
                                sps[h][:, 512 * j2:512 * j2 + 512],
                                kt[qo:qo + 64, i, 128 * k_t:128 * k_t + 128],
                                qt[qo:qo + 64, i, 512 * qc:512 * qc + 512],
                                start=True, stop=True,
                            )
                    if kn["interleave_ctx"] and blk > 0:
                        for h, qo in heads:
                            emit_ctx(h, 2 + blk - 1)
                    for h, qo in heads:
                        if (qc, i) in kn["exp_dve"] and blk % 2 == 1:
                            # exp via Schraudolph bit trick on DVE:
                            # fp16 bits of exp(s/8) ~= 184.664*(s + 82.928)
                            # (as int16; bitcast back to fp16 at the P@V use)
                            pt_b = ptp.tile([128, 1024], I16, tag="pti")
                            nc.vector.tensor_scalar(
                                pt_b[:, :], sps[h][:, :], 82.9283, 184.664965,
                                op0=Alu.add, op1=Alu.mult,
                            )
                            pts[h].append((pt_b.bitcast(FP16),
                                           ((None, 0, 512), (None, 512, 512))))
                        else:
                            pt_b = ptp.tile([128, 1024], FP16, tag="pt")
                            nc.scalar.activation(pt_b[:, :], sps[h][:, :], Exp, scale=0.125)
                            pts[h].append((pt_b, ((None, 0, 512), (None, 512, 512))))
                # remaining ctx accumulation per head
                for h, qo in heads:
                    emit_ctx(h, len(pts[h]))
                # normalize both heads: l rows to SBUF, one recip, broadcast,
                # scale each head's PSUM ctx into ctxn
                lc_eng = (nc.scalar.copy if (qc, i) in kn["lcopy_scalar"]
                          else nc.vector.tensor_copy)
                for n, (h, qo) in enumerate(heads):
                    r_sb = wk1.tile([1, 512], F32, tag="rrow")
                    if kn["recip_psum"]:
                        nc.vector.reciprocal_approx_fast(r_sb[:, :], ctxs[h][64:65, :])
                    else:
                        l_sb = wk1.tile([1, 512], F32, tag="lrow")
                        lc_eng(l_sb[:, :], ctxs[h][64:65, :])
                        nc.vector.reciprocal_approx_fast(r_sb[:, :], l_sb[:, :])
                    rb = wk1.tile([64, 512], F32, tag="rb")
                    nc.gpsimd.partition_broadcast(rb[:, :], r_sb[:1, :], channels=64)
                    nc.vector.tensor_mul(
                        ctxn[qo:qo + 64, i, 512 * qc:512 * qc + 512],
                        ctxs[h][0:64, :], rb[:, :],
                    )

            def emit_oslot(qc, st):
                mode = kn["drain"].get(qc, "dve")
                if True:
                    ob = wk2.tile([128, 2, 512], FP16, tag="ob")
                    for oc in range(2):
                        pso = ps_o.tile([128, 512], F32, tag="po")
                        for dp in range(2):
                            nc.tensor.matmul(
                                pso[:, :],
                                ctxn[:, dp, 128 * st:128 * st + 128],
                                wo[:, dp, 512 * oc:512 * oc + 512],
                                start=(dp == 0), stop=(dp == 1),
                            )
                        use_scalar = (
                            (mode == "split" and oc == 1)
                            or mode == "scalar"
                            or (mode == "alt" and (st + oc) % 2 == 1)
                        )
                        if use_scalar:
                            nc.scalar.copy(ob[:, oc, :], pso[:, :])
                        else:
                            nc.vector.tensor_copy(ob[:, oc, :], pso[:, :])
                    nc.sync.dma_start(
                        out_d[128 * st:128 * st + 128, :],
                        ob.rearrange("p a b -> p (a b)"),
                    )

            for op in kn["order"]:
                if op[0] == "qk":
                    _, which, sc = op
                    emit_qk(wq if which == "q" else wk,
                            qt if which == "q" else kt, sc)
                elif op[0] == "v":
                    emit_v(op[1])
                elif op[0] == "pair":
                    emit_head_pair(op[1], op[2])
                elif op[0] == "pairf":
                    _, qc_, i_, oq_, ost_ = op
                    emit_head_pair(qc_, i_,
                                   filler=lambda: emit_oslot(oq_, ost_))
                elif op[0] == "oproj":
                    for st in range(4 * op[1], 4 * op[1] + 4):
                        emit_oslot(op[1], st)
                elif op[0] == "oslot":
                    emit_oslot(op[1], op[2])
    nc.compile()
    return nc


_NC = None


def _get_nc():
    global _NC
    if _NC is None:
        _NC = _build()
    return _NC


def _in_maps(x, wq, wk, wv, wo):
    xts = [np.ascontiguousarray(x[b].T).astype(np.float16) for b in range(B)]
    in_maps = []
    for c in range(N_CORES):
        b, g = c // 4, c % 4
        sl = slice(GD * g, GD * g + GD)
        in_maps.append({
            "xt": xts[b],
            "wqT": np.ascontiguousarray(wq[sl, :].T).astype(np.float16),
            "wkT": np.ascontiguousarray(wk[sl, :].T).astype(np.float16),
            "wvT": np.ascontiguousarray(wv[sl, :].T).astype(np.float16),
            "woT": np.ascontiguousarray(wo[:, sl].T).astype(np.float16),
        })
    return in_maps


def kernel(**inputs):
    x = np.asarray(inputs["inputs"], dtype=np.float32)
    wq = np.asarray(inputs["Wq"], dtype=np.float32)
    wk = np.asarray(inputs["Wk"], dtype=np.float32)
    wv = np.asarray(inputs["Wv"], dtype=np.float32)
    wo = np.asarray(inputs["Wo"], dtype=np.float32)
    bo = np.asarray(inputs["bo"], dtype=np.float32)

    nc = _get_nc()
    res = run_bass_kernel_spmd(nc, _in_maps(x, wq, wk, wv, wo),
                               core_ids=list(range(N_CORES)))
    out = np.empty((B, S, D), np.float32)
    for b in range(B):
        acc = res.results[4 * b + 0]["out"].astype(np.float32)
        for g in range(1, 4):
            acc = acc + res.results[4 * b + g]["out"].astype(np.float32)
        out[b] = acc + bo
    return out


# revision 18
# speedup vs baseline: 1.1577x; 1.0049x over previous
"""Causal multi-head attention (B=2, S=2048, D=1024, H=16, HD=64) on 8 NeuronCores.

Sharding: core c = 4*b + g handles batch b (2-way data parallel) and head
group g (4-way tensor parallel over the 16 heads, 4 heads per core).
Each core computes its 4 heads' attention plus the partial output
projection (columns of Wo for its heads); the host sums the 4 partials
per batch ("row-parallel" reduction) to produce the full output.

Device layout notes:
  - All I/O and matmul operands are fp16: halves DMA bytes (the TimelineSim
    DMA_ENGINES resource is aggregate-bandwidth serialized) and keeps every
    matmul at 1.0 cycles/row regardless of moving width.
  - Input DMAs are merged into few large transfers (HWDGE charges ~630ns of
    a single serialized resource per dma_start); output stages into
    [128, 2, 512] tiles so each 128-row band is one DMA.
  - X is fed transposed (xt = X[b].T, [D,S]) so the d-contraction of the
    QKV projections has d on SBUF partitions; Q,K are produced transposed
    so scores are computed transposed (S^T[k,q]) and P@V needs no
    transposes; V carries a ones column so P@V also yields the softmax
    denominator; softmax skips max-subtraction (scores/8 ~ N(0,1)).
  - exp runs on the scalar engine only (the sole exp engine, ~75us of an
    ~115us-PE kernel): that queue is kept free of other work mid-kernel.
  - chunks 0 and 1 of the attention run inside the DMA-bound projection
    window; the exp-heavy chunks 3 and 2 run interleaved afterwards.
"""

import numpy as np

import concourse.mybir as mybir
from concourse import bacc
from concourse.tile import TileContext
from concourse.bass_utils import run_bass_kernel_spmd
from concourse.masks import make_identity, make_upper_triangular

F32 = mybir.dt.float32
FP16 = mybir.dt.float16
I16 = mybir.dt.int16
Exp = mybir.ActivationFunctionType.Exp
Alu = mybir.AluOpType

B, S, D, H, HD = 2, 2048, 1024, 16, 64
GH = 4            # heads per core
GD = GH * HD      # 256 features per core
N_CORES = 8

# Schedule / engine-assignment knobs (overridable for experiments).
KNOBS = dict(
    # emission order: projections with chunk-0/1 attention folded in, then
    # the exp-heavy chunk-2/3 pairs with output-projection slots woven in
    # at pair boundaries (PE filler while the scalar engine drains exps)
    order=[
        ("qk", "q", 0), ("qk", "q", 1), ("v", 0), ("v", 1), ("qk", "k", 0),
        ("pair", 0, 0), ("pair", 0, 1),
        ("qk", "q", 2), ("qk", "k", 1), ("v", 2),
        ("pair", 1, 0),
        ("qk", "q", 3), ("qk", "k", 2), ("v", 3), ("qk", "k", 3),
        ("oslot", 0, 0), ("pair", 1, 1),
        ("oslot", 0, 1), ("pair", 3, 0),
        ("oslot", 0, 2), ("oslot", 1, 4), ("pair", 2, 0),
        ("oslot", 0, 3), ("oslot", 1, 5), ("pair", 3, 1),
        ("oslot", 1, 6), ("oslot", 1, 7), ("pair", 2, 1),
        ("oproj", 3), ("oproj", 2),
    ],
    # which oproj chunks split their drains across DVE+scalar: {qc: mode}
    # mode: "dve" (all DVE), "split" (oc0 DVE, oc1 scalar), "scalar", "alt"
    drain={0: "dve", 3: "scalar", 2: "split", 1: "split"},
    pt_bufs=10,
    ps_a_bufs=2,
    pack_order="21",   # small diagonal pack first: its short exps free the
                       # score PSUM ring faster at pair starts
    dma_plan="fine",
    # pairs whose l-row copies go to the scalar engine (set of (qc,i))
    lcopy_scalar=set(),
    recip_psum=False,   # reciprocal reads the PSUM l-row directly
    # pairs whose odd block-exps run on DVE via the Schraudolph bit trick
    exp_dve=set(),
    interleave_ctx=False,  # weave ctx MMs between block score MMs
    mask_mm=False,      # accumulate -1e3 mask via PE instead of tri-muls
    warmup=0,           # dummy PE matmuls at t=0 to warm the pstate ramp
    dma_first_fine=False,
    dma_per_oc={2},     # oproj chunks whose output DMA fires per 512-half
)


def _build(knobs=None):
    kn = dict(KNOBS)
    if knobs:
        kn.update(knobs)

    nc = bacc.Bacc("TRN2", target_bir_lowering=False, name="mha_tp")
    xt_d = nc.dram_tensor("xt", [D, S], FP16, kind="ExternalInput")
    wq_d = nc.dram_tensor("wqT", [D, GD], FP16, kind="ExternalInput")
    wk_d = nc.dram_tensor("wkT", [D, GD], FP16, kind="ExternalInput")
    wv_d = nc.dram_tensor("wvT", [D, GD], FP16, kind="ExternalInput")
    wo_d = nc.dram_tensor("woT", [GD, D], FP16, kind="ExternalInput")
    out_d = nc.dram_tensor("out", [S, D], FP16, kind="ExternalOutput")

    with TileContext(nc) as tc:
        with (
            tc.tile_pool(name="per", bufs=1) as per,
            tc.tile_pool(name="pt", bufs=kn["pt_bufs"]) as ptp,
            tc.tile_pool(name="wk1", bufs=1) as wk1,
            tc.tile_pool(name="wk2", bufs=6) as wk2,
            tc.tile_pool(name="ps_a", bufs=kn["ps_a_bufs"], space="PSUM") as ps_a,
            tc.tile_pool(name="ps_o", bufs=2, space="PSUM") as ps_o,
            tc.tile_pool(name="ps_c", bufs=2, space="PSUM") as ps_c,
        ):
            xt = per.tile([128, 8, S], FP16)       # X^T, d-tile major
            wo = per.tile([128, 2, D], FP16)       # Wo^T for our head cols
            qt = per.tile([128, 2, S], FP16)       # Q^T (2 heads per tile)
            kt = per.tile([128, 2, S], FP16)
            vaug = per.tile([128, 16, 4 * (HD + 1)], FP16)  # V + ones col per head
            ctxn = per.tile([128, 2, S], FP16)     # normalized ctx^T
            tri = per.tile([128, 128], FP16)       # tri[kk,c]=1 iff kk<=c
            wq = per.tile([128, 8, GD], FP16)
            wk = per.tile([128, 8, GD], FP16)
            wv = per.tile([128, 8, GD], FP16)

            make_upper_triangular(nc, tri[:, :], val=1.0, diag=True)
            if kn["mask_mm"]:
                ident = per.tile([128, 128], FP16)
                trim = per.tile([128, 128], FP16)
                make_identity(nc, ident[:, :])
                # trim[kk, c] = -1e3 where kk > c (masked region), else 0:
                # (tri - 1) * 1e3
                nc.vector.tensor_scalar(
                    trim[:, :], tri[:, :], -1.0, 1e3,
                    op0=Alu.add, op1=Alu.mult,
                )
            if kn["warmup"]:
                wps = ps_o.tile([128, 512], F32, tag="po")
                for wi in range(kn["warmup"]):
                    nc.tensor.matmul(wps[:, 0:128], tri[:, :], tri[:, :],
                                     start=(wi == 0),
                                     stop=(wi == kn["warmup"] - 1))

            # ---- input DMA waves. dram rows are d-features: row 128*a + p
            # maps to SBUF partition p, d-tile slot a.
            def dview(t, r0, r1, c0, c1):
                return t[r0:r1, c0:c1].rearrange("(a p) c -> p a c", p=128)

            if kn["dma_plan"] == "fine":
                if kn["dma_first_fine"]:
                    nc.sync.dma_start(wq[:, 0:1, :], dview(wq_d, 0, 128, 0, GD))
                    nc.scalar.dma_start(xt[:, 0:1, 0:512], dview(xt_d, 0, 128, 0, 512))
                    nc.sync.dma_start(wq[:, 1:2, :], dview(wq_d, 128, 256, 0, GD))
                    nc.scalar.dma_start(xt[:, 1:2, 0:512], dview(xt_d, 128, 256, 0, 512))
                else:
                    nc.sync.dma_start(wq[:, 0:2, :], dview(wq_d, 0, 256, 0, GD))
                    nc.scalar.dma_start(xt[:, 0:2, 0:512], dview(xt_d, 0, 256, 0, 512))
                nc.sync.dma_start(wq[:, 2:5, :], dview(wq_d, 256, 640, 0, GD))
                nc.scalar.dma_start(xt[:, 2:5, 0:512], dview(xt_d, 256, 640, 0, 512))
                nc.sync.dma_start(wq[:, 5:8, :], dview(wq_d, 640, 1024, 0, GD))
                nc.scalar.dma_start(xt[:, 5:8, 0:512], dview(xt_d, 640, 1024, 0, 512))
                nc.sync.dma_start(xt[:, 0:4, 512:1024], dview(xt_d, 0, 512, 512, 1024))
                nc.scalar.dma_start(xt[:, 4:8, 512:1024], dview(xt_d, 512, 1024, 512, 1024))
                nc.sync.dma_start(wv[:, :, :], dview(wv_d, 0, 1024, 0, GD))
                nc.scalar.dma_start(wk[:, :, :], dview(wk_d, 0, 1024, 0, GD))
                nc.scalar.dma_start(xt[:, :, 1024:1536], dview(xt_d, 0, 1024, 1024, 1536))
                nc.sync.dma_start(xt[:, :, 1536:2048], dview(xt_d, 0, 1024, 1536, 2048))
                nc.sync.dma_start(wo[:, :, :], dview(wo_d, 0, 256, 0, D))
            else:
                nc.sync.dma_start(wq[:, 0:2, :], dview(wq_d, 0, 256, 0, GD))
                nc.scalar.dma_start(xt[:, 0:2, 0:512], dview(xt_d, 0, 256, 0, 512))
                nc.sync.dma_start(wq[:, 2:8, :], dview(wq_d, 256, 1024, 0, GD))
                nc.scalar.dma_start(xt[:, 2:8, 0:512], dview(xt_d, 256, 1024, 0, 512))
                nc.sync.dma_start(wk[:, :, :], dview(wk_d, 0, 1024, 0, GD))
                nc.scalar.dma_start(xt[:, :, 512:1024], dview(xt_d, 0, 1024, 512, 1024))
                nc.sync.dma_start(wv[:, :, :], dview(wv_d, 0, 1024, 0, GD))
                nc.scalar.dma_start(xt[:, :, 1024:1536], dview(xt_d, 0, 1024, 1024, 1536))
                nc.sync.dma_start(xt[:, :, 1536:2048], dview(xt_d, 0, 1024, 1536, 2048))
                nc.sync.dma_start(wo[:, :, :], dview(wo_d, 0, 256, 0, D))

            def emit_qk(w_t, dst, sc):
                for dp in range(2):
                    ps = ps_a.tile([128, 512], F32, tag="blk")
                    for dt in range(8):
                        nc.tensor.matmul(
                            ps[:, :],
                            w_t[:, dt, 128 * dp:128 * dp + 128],
                            xt[:, dt, 512 * sc:512 * sc + 512],
                            start=(dt == 0), stop=(dt == 7),
                        )
                    nc.vector.tensor_copy(dst[:, dp, 512 * sc:512 * sc + 512], ps[:, :])

            def emit_v(sc):
                for st in range(4 * sc, 4 * sc + 4):
                    psv = ps_a.tile([128, 256], F32, tag="blk")
                    for dt in range(8):
                        nc.tensor.matmul(
                            psv[:, :],
                            xt[:, dt, 128 * st:128 * st + 128],
                            wv[:, dt, :],
                            start=(dt == 0), stop=(dt == 7),
                        )
                    v_dst = vaug[:, st, :].rearrange("p (h c) -> p h c", c=HD + 1)
                    nc.vector.tensor_copy(
                        v_dst[:, :, 0:HD],
                        psv.rearrange("p (h c) -> p h c", c=HD),
                    )
                    # ones column: x*0+1 through DVE so the write is rounded
                    nc.vector.tensor_scalar(
                        v_dst[:, :, HD:HD + 1], psv[:, 0:4], 0.0, 1.0,
                        op0=Alu.mult, op1=Alu.add,
                    )

            def emit_head_pair(qc, i, filler=None):
                """Heads hA=2i (PE rows 0-63) and hB=2i+1 (rows 64-127): their
                score matmuls are emitted alternating so the two heads' chains
                pipeline through exp/ctx independently. With interleave_ctx,
                each block's P@V accumulation is emitted one block behind the
                scores so the PE has work while exps drain."""
                hA, hB = 2 * i, 2 * i + 1
                heads = ((hA, 0), (hB, 64))
                ctxs = {}
                pts = {h: [] for h, _ in heads}
                packs = [
                    (896, ((0, 0, 512), (1, 512, 384))),
                    (384, ((3, 0, 128), (2, 128, 256))),
                ]
                if kn["pack_order"] == "21":
                    packs = packs[::-1]
                for h, qo in heads:
                    ctx_t = ps_c.tile([65, 512], F32, tag="ctx")
                    ctxs[h] = ctx_t

                n_ctx = {h: 2 * (2 + 2 * qc) for h, _ in heads}
                ctx_done = {h: 0 for h, _ in heads}

                def emit_ctx(h, upto):
                    ctx = ctxs[h]
                    jobs = []
                    for bi, (pt_t, parts) in enumerate(pts[h][:upto]):
                        for pj, (j, o, w) in enumerate(parts):
                            if bi < 2:          # diagonal strip tiles
                                k_t, co = 4 * qc + j, 128 * j
                            else:               # full block tiles
                                k_t, co = 2 * (bi - 2) + pj, 0
                            jobs.append((pt_t, k_t, o, w, co))
                    total = n_ctx[h]
                    for n in range(ctx_done[h], len(jobs)):
                        pt_t, k_t, o, w, co = jobs[n]
                        nc.tensor.matmul(
                            ctx[:, co:co + w],
                            vaug[:, k_t, 65 * h:65 * h + 65],
                            pt_t[:, o:o + w],
                            start=(n == 0), stop=(n == total - 1),
                            skip_group_check=True,
                        )
                    ctx_done[h] = len(jobs)

                # diagonal strips
                for width, parts in packs:
                    sps = {}
                    for h, qo in heads:
                        sp_t = ps_a.tile([128, 1024], F32, tag="blk")
                        sps[h] = sp_t
                    for j, o, w in parts:
                        k_t = 4 * qc + j
                        for h, qo in heads:
                            nc.tensor.matmul(
                                sps[h][:, o:o + w],
                                kt[qo:qo + 64, i, 128 * k_t:128 * k_t + 128],
                                qt[qo:qo + 64, i, 512 * qc + 128 * j:512 * qc + 128 * j + w],
                                start=True, stop=True,
                            )
                    for h, qo in heads:
                        pt_p = ptp.tile([128, 1024], FP16, tag="pt")
                        nc.scalar.activation(pt_p[:, :width], sps[h][:, :width], Exp, scale=0.125)
                        for ii, (j, o, w) in enumerate(parts):
                            eng = nc.vector if ii == 0 else nc.gpsimd
                            eng.tensor_mul(
                                pt_p[:, o:o + 128], pt_p[:, o:o + 128], tri[:, :]
                            )
                        pts[h].append((pt_p, parts))
                if filler is not None:
                    filler()
                # full blocks (2 k-tiles per tile), pairwise
                for blk in range(2 * qc):
                    sps = {}
                    for h, qo in heads:
                        sp_t = ps_a.tile([128, 1024], F32, tag="blk")
                        sps[h] = sp_t
                    for j2 in range(2):
                        k_t = 2 * blk + j2
                        for h, qo in heads:
                            nc.tensor.matmul(
                                sps[h][:, 512 * j2:512 * j2 + 512],
                                kt[qo:qo + 64, i, 128 * k_t:128 * k_t + 128],
                                qt[qo:qo + 64, i, 512 * qc:512 * qc + 512],
                                start=True, stop=True,
                            )
                    if kn["interleave_ctx"] and blk > 0:
                        for h, qo in heads:
                            emit_ctx(h, 2 + blk - 1)
                    for h, qo in heads:
                        if (qc, i) in kn["exp_dve"] and blk % 2 == 1:
                            # exp via Schraudolph bit trick on DVE:
                            # fp16 bits of exp(s/8) ~= 184.664*(s + 82.928)
                            # (as int16; bitcast back to fp16 at the P@V use)
                            pt_b = ptp.tile([128, 1024], I16, tag="pti")
                            nc.vector.tensor_scalar(
                                pt_b[:, :], sps[h][:, :], 82.9283, 184.664965,
                                op0=Alu.add, op1=Alu.mult,
                            )
                            pts[h].append((pt_b.bitcast(FP16),
                                           ((None, 0, 512), (None, 512, 512))))
                        else:
                            pt_b = ptp.tile([128, 1024], FP16, tag="pt")
                            nc.scalar.activation(pt_b[:, :], sps[h][:, :], Exp, scale=0.125)
                            pts[h].append((pt_b, ((None, 0, 512), (None, 512, 512))))
                # remaining ctx accumulation per head
                for h, qo in heads:
                    emit_ctx(h, len(pts[h]))
                # normalize both heads: l rows to SBUF, one recip, broadcast,
                # scale each head's PSUM ctx into ctxn
                lc_eng = (nc.scalar.copy if (qc, i) in kn["lcopy_scalar"]
                          else nc.vector.tensor_copy)
                for n, (h, qo) in enumerate(heads):
                    r_sb = wk1.tile([1, 512], F32, tag="rrow")
                    if kn["recip_psum"]:
                        nc.vector.reciprocal_approx_fast(r_sb[:, :], ctxs[h][64:65, :])
                    else:
                        l_sb = wk1.tile([1, 512], F32, tag="lrow")
                        lc_eng(l_sb[:, :], ctxs[h][64:65, :])
                        nc.vector.reciprocal_approx_fast(r_sb[:, :], l_sb[:, :])
                    rb = wk1.tile([64, 512], F32, tag="rb")
                    nc.gpsimd.partition_broadcast(rb[:, :], r_sb[:1, :], channels=64)
                    nc.vector.tensor_mul(
                        ctxn[qo:qo + 64, i, 512 * qc:512 * qc + 512],
                        ctxs[h][0:64, :], rb[:, :],
                    )

            def emit_oslot(qc, st):
                mode = kn["drain"].get(qc, "dve")
                if True:
                    ob = wk2.tile([128, 2, 512], FP16, tag="ob")
                    for oc in range(2):
                        pso = ps_o.tile([128, 512], F32, tag="po")
                        for dp in range(2):
                            nc.tensor.matmul(
                                pso[:, :],
                                ctxn[:, dp, 128 * st:128 * st + 128],
                                wo[:, dp, 512 * oc:512 * oc + 512],
                                start=(dp == 0), stop=(dp == 1),
                            )
                        use_scalar = (
                            (mode == "split" and oc == 1)
                            or mode == "scalar"
                            or (mode == "alt" and (st + oc) % 2 == 1)
                        )
                        if use_scalar:
                            nc.scalar.copy(ob[:, oc, :], pso[:, :])
                        else:
                            nc.vector.tensor_copy(ob[:, oc, :], pso[:, :])
                        if qc in kn["dma_per_oc"]:
                            nc.sync.dma_start(
                                out_d[128 * st:128 * st + 128,
                                      512 * oc:512 * oc + 512],
                                ob[:, oc, :],
                            )
                    if qc not in kn["dma_per_oc"]:
                        nc.sync.dma_start(
                            out_d[128 * st:128 * st + 128, :],
                            ob.rearrange("p a b -> p (a b)"),
                        )

            for op in kn["order"]:
                if op[0] == "qk":
                    _, which, sc = op
                    emit_qk(wq if which == "q" else wk,
                            qt if which == "q" else kt, sc)
                elif op[0] == "v":
                    emit_v(op[1])
                elif op[0] == "pair":
                    emit_head_pair(op[1], op[2])
                elif op[0] == "pairf":
                    _, qc_, i_, oq_, ost_ = op
                    emit_head_pair(qc_, i_,
                                   filler=lambda: emit_oslot(oq_, ost_))
                elif op[0] == "oproj":
                    for st in range(4 * op[1], 4 * op[1] + 4):
                        emit_oslot(op[1], st)
                elif op[0] == "oslot":
                    emit_oslot(op[1], op[2])
    nc.compile()
    return nc


_NC = None


def _get_nc():
    global _NC
    if _NC is None:
        _NC = _build()
    return _NC


def _in_maps(x, wq, wk, wv, wo):
    xts = [np.ascontiguousarray(x[b].T).astype(np.float16) for b in range(B)]
    in_maps = []
    for c in range(N_CORES):
        b, g = c // 4, c % 4
        sl = slice(GD * g, GD * g + GD)
        in_maps.append({
            "xt": xts[b],
            "wqT": np.ascontiguousarray(wq[sl, :].T).astype(np.float16),
            "wkT": np.ascontiguousarray(wk[sl, :].T).astype(np.float16),
            "wvT": np.ascontiguousarray(wv[sl, :].T).astype(np.float16),
            "woT": np.ascontiguousarray(wo[:, sl].T).astype(np.float16),
        })
    return in_maps


def kernel(**inputs):
    x = np.asarray(inputs["inputs"], dtype=np.float32)
    wq = np.asarray(inputs["Wq"], dtype=np.float32)
    wk = np.asarray(inputs["Wk"], dtype=np.float32)
    wv = np.asarray(inputs["Wv"], dtype=np.float32)
    wo = np.asarray(inputs["Wo"], dtype=np.float32)
    bo = np.asarray(inputs["bo"], dtype=np.float32)

    nc = _get_nc()
    res = run_bass_kernel_spmd(nc, _in_maps(x, wq, wk, wv, wo),
                               core_ids=list(range(N_CORES)))
    out = np.empty((B, S, D), np.float32)
    for b in range(B):
        acc = res.results[4 * b + 0]["out"].astype(np.float32)
        for g in range(1, 4):
            acc = acc + res.results[4 * b + g]["out"].astype(np.float32)
        out[b] = acc + bo
    return out
